# revision 40
# baseline (speedup 1.0000x reference)
"""Trainium2 Bass kernel for nn_BertEncoder_403726926494.

Reference computation (per batch element):
  - ragged sentence extraction from hidden_states, masked-softmax attention
    pooling per sentence with W_doc            -> doc_pooled [B, D, H]
  - query extraction (rows 1..32), masked-softmax pooling with W_query
    broadcast over D                           -> q_bcast   [B, D, H]

Device strategy (SPMD, one program on 8 cores, 8 batch elements per core):
  - Dense merged token packing: each core receives ONE fp16 stream holding
    its 8 examples' doc-sentence tokens, then (from the global boundary row
    DSTOP, a program constant) a copy of their query tokens.  Token t lands
    on SBUF partition t%128 of chunk t//128; 12 chunks cover everything.
    A trailing ones-column (col 768) rides along for the denominators.
  - Per chunk: a fused DVE scalar_tensor_tensor computes xw = x*W and the
    per-token score s_t (fp32) in one pass (1x rate: the DVE accumulator
    path disables the 16-bit 2x mode); a few mid-stream chunks instead use
    a 2x-rate tensor_tensor with the reduce offloaded to ACT.  The chunk
    holding the doc/query boundary uses a host-built per-partition W tile
    (wd rows below the boundary, wq above).  ONE ACT Exp over a host-built
    fp8 log-mask (0 where token t belongs to pooling column m, -4096
    elsewhere) with bias=s_t yields at[t,m] = exp(s_t)*onehot[t,m]; the
    boundary chunk runs a second small Exp for its query columns.
  - Doc pooling columns: 8 examples x 16 sentences = exactly 128 PE columns,
    so ONE PSUM accumulation group [128, 769] collects num|den for every
    sentence of the core; query chunks accumulate into a second group
    [32, 769] (8 cols used).  fp16 matmuls, fp32 PSUM.
  - out = num / den; the den epsilon rides the mask (a reserved pad row per
    section carries EPS_LOG on every column), so empty sentences come out
    exactly 0.  Results leave as fp16 and are scattered on the host.
    b_doc / b_query shift every score in a softmax segment equally, so
    they cancel and are ignored.
"""

import numpy as np
import ml_dtypes

B, L, H = 64, 512, 768
D, S, Q = 16, 64, 32
NCORES = 8
EPB = 8  # examples per core
NEG = -4096.0  # exp(NEG + s) == 0
EPS_LOG = -8.0  # reserved pad rows carry this: den >= exp(-8), fp16-safe
DEN_EPS = 1.0e-30

F16 = np.float16
MASK_F8 = True
F8 = ml_dtypes.float8_e5m2 if MASK_F8 else np.float16

# Score-pass engine plan: "stt" = fused DVE op (default); "tta" = 2x-rate
# DVE tensor_tensor + ACT Copy-accum reduce (ACT slack absorbs ~3 reduces).
# GpSimd stays off the stream entirely: it shares SBUF ports with DVE.
SCORE_PLAN = {4: "tta", 6: "tta", 8: "tta"}


def _score_mode(c):
    return SCORE_PLAN.get(c, "stt")

_compiled: dict = {}


def _build(NTD, NT, BOUND):
    """Build + compile the SPMD Bass program for the given chunk geometry.

    NTD: chunks containing doc rows (the last of them is the mixed chunk),
    NT: total chunks, BOUND: boundary partition inside chunk NTD-1.
    """
    from contextlib import ExitStack

    import concourse.bacc as bacc
    import concourse.tile as tile
    from concourse import mybir

    f32 = mybir.dt.float32
    f16 = mybir.dt.float16
    f8 = mybir.dt.float8e5 if MASK_F8 else mybir.dt.float16
    MULT = mybir.AluOpType.mult
    ADD = mybir.AluOpType.add
    EXP = mybir.ActivationFunctionType.Exp
    COPY = mybir.ActivationFunctionType.Copy

    W = H + 1  # 769: H data cols + ones col
    MIX = NTD - 1  # the mixed doc/query chunk

    nc = bacc.Bacc(
        "TRN2", target_bir_lowering=False, debug=False, num_devices=NCORES
    )
    xd = nc.dram_tensor("xd", [128, NT, W], f16, kind="ExternalInput").ap()
    mask8 = nc.dram_tensor("mask8", [128, NT, 128], f8, kind="ExternalInput").ap()
    maskq2 = nc.dram_tensor("maskq2", [128, 32], f8, kind="ExternalInput").ap()
    wbd = nc.dram_tensor("wbd", [128, H], f16, kind="ExternalInput").ap()
    wbq = nc.dram_tensor("wbq", [128, H], f16, kind="ExternalInput").ap()
    wbm = nc.dram_tensor("wbm", [128, H], f16, kind="ExternalInput").ap()
    doc_out = nc.dram_tensor("doc_out", [128, H], f16, kind="ExternalOutput").ap()
    q_out = nc.dram_tensor("q_out", [EPB, H], f16, kind="ExternalOutput").ap()

    with tile.TileContext(nc) as tc, ExitStack() as ctx:
        const = ctx.enter_context(tc.tile_pool(name="const", bufs=1))
        atp = ctx.enter_context(tc.tile_pool(name="atp", bufs=3))
        scrp = ctx.enter_context(tc.tile_pool(name="scr", bufs=4))
        outp = ctx.enter_context(tc.tile_pool(name="outp", bufs=2))
        smallp = ctx.enter_context(tc.tile_pool(name="smallp", bufs=4))
        nump = ctx.enter_context(tc.tile_pool(name="nump", bufs=1, space="PSUM"))
        qnump = ctx.enter_context(tc.tile_pool(name="qnump", bufs=1, space="PSUM"))

        x = const.tile([128, NT, W], f16)
        mask_t = const.tile([128, NT, 128], f8)
        maskq2_t = const.tile([128, 32], f8)
        wb_d = const.tile([128, H], f16)
        wb_q = const.tile([128, H], f16)
        wb_m = const.tile([128, H], f16)
        scol = const.tile([128, NT], f32)
        s2 = const.tile([128, H], f16)  # dummy out for ACT accum reduce

        # ---- input DMAs: singles up front alternating rings (early chunk
        # semaphores), pairs behind; boundary-chunk extras ride late.
        def xg(ring, a, b):
            a, b = min(a, NT), min(b, NT)
            if b > a:
                ring.dma_start(out=x[:, a:b, :], in_=xd[:, a:b, :])

        nc.sync.dma_start(out=wb_d[:], in_=wbd[:])
        xg(nc.scalar, 0, 1)
        xg(nc.sync, 1, 2)
        xg(nc.scalar, 2, 3)
        nc.scalar.dma_start(out=mask_t[:, 0:2, :], in_=mask8[:, 0:2, :])
        xg(nc.sync, 3, 4)
        xg(nc.scalar, 4, 5)
        xg(nc.sync, 5, 6)
        nc.scalar.dma_start(out=mask_t[:, 2:NT, :], in_=mask8[:, 2:NT, :])
        xg(nc.sync, 6, 8)
        xg(nc.scalar, 8, 9)
        xg(nc.sync, 9, 10)
        nc.scalar.dma_start(out=wb_m[:], in_=wbm[:])
        nc.scalar.dma_start(out=maskq2_t[:], in_=maskq2[:])
        xg(nc.sync, 10, 11)
        nc.scalar.dma_start(out=wb_q[:], in_=wbq[:])
        xg(nc.sync, 11, NT)

        numg = nump.tile([128, 1024], f32, tag="num", name="num")
        qnumg = qnump.tile([32, 1024], f32, tag="qnum", name="qnum")

        def emit_chunk(c):
            if c < MIX:
                wb = wb_d
            elif c == MIX:
                wb = wb_m
            else:
                wb = wb_q

            xw = scrp.tile([128, H], f16, tag="xw", name=f"xw{c}")
            mode = _score_mode(c)
            if mode == "tta":
                nc.vector.tensor_tensor(
                    out=xw[:], in0=x[:, c, 0:H], in1=wb[:], op=MULT
                )
                nc.scalar.activation(
                    s2[:], xw[:], COPY, bias=0.0, scale=1.0,
                    accum_out=scol[:, c : c + 1],
                )
            else:
                nc.vector.scalar_tensor_tensor(
                    out=xw[:], in0=x[:, c, 0:H], scalar=1.0, in1=wb[:],
                    op0=MULT, op1=MULT, accum_out=scol[:, c : c + 1],
                )

            if c <= MIX:  # doc side
                at = atp.tile([128, 128], f16, tag="at", name=f"at{c}")
                nc.scalar.activation(
                    at[:], mask_t[:, c, :], EXP,
                    bias=scol[:, c : c + 1], scale=1.0,
                )
                nc.tensor.matmul(
                    numg[:, 0:512], at[:], x[:, c, 0:512],
                    start=c == 0, stop=c == MIX,
                )
                nc.tensor.matmul(
                    numg[:, 512:W], at[:], x[:, c, 512:W],
                    start=c == 0, stop=c == MIX,
                )
            if c >= MIX:  # query side
                msrc = maskq2_t[:] if c == MIX else mask_t[:, c, 0:32]
                atq = atp.tile([128, 128], f16, tag="at", name=f"atq{c}")
                nc.scalar.activation(
                    atq[:, 0:32], msrc, EXP,
                    bias=scol[:, c : c + 1], scale=1.0,
                )
                nc.tensor.matmul(
                    qnumg[0:32, 0:512], atq[:, 0:32], x[:, c, 0:512],
                    start=c == MIX, stop=c == NT - 1,
                )
                nc.tensor.matmul(
                    qnumg[0:32, 512:W], atq[:, 0:32], x[:, c, 512:W],
                    start=c == MIX, stop=c == NT - 1,
                )

        for c in range(NT):
            emit_chunk(c)

        # ---- doc finish: out = num / den (eps rides the mask) ----
        rec = smallp.tile([128, 1], f32, tag="rec", name="rec")
        nc.vector.reciprocal(rec[:], numg[:, H : H + 1])
        do = outp.tile([128, H], f16, tag="do", name="do")
        nc.vector.tensor_scalar(
            out=do[:, 0:384], in0=numg[:, 0:384], scalar1=rec[:, 0:1],
            scalar2=None, op0=MULT,
        )
        nc.scalar.activation(
            do[:, 384:H], numg[:, 384:H], COPY, bias=0.0, scale=rec[:, 0:1]
        )
        nc.sync.dma_start(out=doc_out[:, 0:384], in_=do[:, 0:384])
        nc.sync.dma_start(out=doc_out[:, 384:H], in_=do[:, 384:H])

        # ---- query finish ----
        qrec = smallp.tile([EPB, 1], f32, tag="qrec", name="qrec")
        nc.vector.reciprocal(qrec[:], qnumg[0:EPB, H : H + 1])
        qo = outp.tile([EPB, H], f16, tag="qo", name="qo")
        nc.vector.tensor_scalar(
            out=qo[:, 0:384], in0=qnumg[0:EPB, 0:384], scalar1=qrec[:, 0:1],
            scalar2=None, op0=MULT,
        )
        nc.scalar.activation(
            qo[:, 384:H], qnumg[0:EPB, 384:H], COPY, bias=0.0,
            scale=qrec[:, 0:1],
        )
        nc.sync.dma_start(out=q_out[:], in_=qo[:])

    nc.compile()
    return nc


def _prepare(query_len, seq_lens):
    """Host-side geometry: joint doc/query balancing + per-core plans."""
    ql = np.asarray(query_len).astype(np.int64)
    sl = np.asarray(seq_lens).astype(np.int64)
    offs = ql[:, None] + 2 + np.cumsum(sl, axis=1) - sl
    doc_tok = sl.sum(axis=1)

    # greedy on total tokens, then swap-repair minimizing max_d + max_q so
    # the merged stream (max_d + 1 eps + max_q + 1 eps) packs tightest
    tot = doc_tok + ql
    order = np.argsort(-tot, kind="stable")
    ex_map = [[] for _ in range(NCORES)]
    ld = np.zeros(NCORES, np.int64)
    lq = np.zeros(NCORES, np.int64)
    for e in order:
        cand = [c for c in range(NCORES) if len(ex_map[c]) < EPB]
        c = min(cand, key=lambda cc: (ld[cc] + lq[cc], cc))
        ex_map[c].append(int(e))
        ld[c] += doc_tok[e]
        lq[c] += ql[e]
    for _ in range(400):
        s0 = ld.max() + lq.max()
        best = None
        for c1 in range(NCORES):
            for c2 in range(c1 + 1, NCORES):
                for i1, e1 in enumerate(ex_map[c1]):
                    for i2, e2 in enumerate(ex_map[c2]):
                        dd = doc_tok[e2] - doc_tok[e1]
                        dq = ql[e2] - ql[e1]
                        nd, nq = ld.copy(), lq.copy()
                        nd[c1] += dd; nq[c1] += dq
                        nd[c2] -= dd; nq[c2] -= dq
                        s = nd.max() + nq.max()
                        if s < s0 and (best is None or s < best[0]):
                            best = (s, c1, c2, i1, i2)
        if best is None:
            break
        _, c1, c2, i1, i2 = best
        e1, e2 = ex_map[c1][i1], ex_map[c2][i2]
        ex_map[c1][i1], ex_map[c2][i2] = e2, e1
        ld[c1] += doc_tok[e2] - doc_tok[e1]; lq[c1] += ql[e2] - ql[e1]
        ld[c2] -= doc_tok[e2] - doc_tok[e1]; lq[c2] -= ql[e2] - ql[e1]

    DSTOP = int(ld.max()) + 1              # doc region incl. 1 eps-pad row
    NTD = -(-DSTOP // 128)                 # chunks containing doc rows
    NT = -(-(DSTOP + int(lq.max()) + 1) // 128)
    BOUND = DSTOP - 128 * (NTD - 1)        # boundary partition in chunk NTD-1

    plan = []
    for c in range(NCORES):
        didx, dsid, qidx, qsid = [], [], [], []
        for k, e in enumerate(ex_map[c]):
            for j in range(D):
                n = int(sl[e, j])
                if n == 0:
                    continue
                o = int(offs[e, j])
                didx.append(np.arange(e * L + o, e * L + o + n))
                dsid.append(np.full(n, 16 * k + j))
            n = int(ql[e])
            qidx.append(np.arange(e * L + 1, e * L + 1 + n))
            qsid.append(np.full(n, k))
        plan.append(
            (
                np.concatenate(didx),
                np.concatenate(dsid),
                np.concatenate(qidx),
                np.concatenate(qsid),
            )
        )
    return ex_map, plan, (NTD, NT, DSTOP, BOUND)


def _stage_core(hs2, plan_c, geom):
    NTD, NT, DSTOP, BOUND = geom
    W = H + 1
    rows = NT * 128
    didx, dsid, qidx, qsid = plan_c

    xs = np.zeros((rows, W), F16)
    xs[:, H] = 1.0  # all rows, incl. padding: den eps rides EPS_LOG
    xs[: len(didx), 0:H] = hs2[didx]
    xs[DSTOP : DSTOP + len(qidx), 0:H] = hs2[qidx]
    xd = np.ascontiguousarray(xs.reshape(NT, 128, W).transpose(1, 0, 2))

    mlin = np.full((rows, 128), NEG, np.float32)
    mlin[DSTOP - 1] = EPS_LOG    # doc den eps (always a pad row)
    mlin[rows - 1] = EPS_LOG     # query den eps
    mlin[np.arange(len(dsid)), dsid] = 0.0
    # query one-hot: pure-q chunks read cols 0:32 of mask8; the mixed
    # chunk's query side uses the separate maskq2
    qrows = DSTOP + np.arange(len(qsid))
    outside = qrows >= NTD * 128
    mlin[qrows[outside], qsid[outside]] = 0.0
    mask8 = np.ascontiguousarray(
        mlin.reshape(NT, 128, 128).transpose(1, 0, 2)
    ).astype(F8)

    mq2 = np.full((128, 32), NEG, np.float32)
    inside = ~outside
    mq2[qrows[inside] - 128 * (NTD - 1), qsid[inside]] = 0.0
    if NTD == NT:  # q eps row falls inside the mixed chunk
        mq2[127] = EPS_LOG
    return xd, mask8, np.ascontiguousarray(mq2).astype(F8)


def kernel(hidden_states, W_doc, b_doc, W_query, b_query, query_len, seq_lens):
    hs = np.asarray(hidden_states, dtype=np.float32)
    hs2 = hs.reshape(B * L, H)
    wdr = np.asarray(W_doc, np.float32).reshape(1, H).astype(F16)
    wqr = np.asarray(W_query, np.float32).reshape(1, H).astype(F16)
    wd = np.ascontiguousarray(np.broadcast_to(wdr, (128, H)))
    wq = np.ascontiguousarray(np.broadcast_to(wqr, (128, H)))

    ex_map, plan, geom = _prepare(query_len, seq_lens)
    NTD, NT, DSTOP, BOUND = geom

    wm = np.empty((128, H), F16)  # mixed-chunk W: wd below BOUND, wq above
    wm[:BOUND] = wdr
    wm[BOUND:] = wqr

    key = (NTD, NT, BOUND)
    nc = _compiled.get(key)
    if nc is None:
        nc = _build(*key)
        _compiled[key] = nc

    in_maps = []
    for c in range(NCORES):
        xd, mask8, mq2 = _stage_core(hs2, plan[c], geom)
        in_maps.append(
            {
                "xd": xd,
                "mask8": mask8,
                "maskq2": mq2,
                "wbd": wd,
                "wbq": wq,
                "wbm": wm,
            }
        )

    from concourse.bass_utils import run_bass_kernel_spmd

    res = run_bass_kernel_spmd(nc, in_maps, list(range(NCORES)))

    doc = np.empty((B, D, H), np.float32)
    qp = np.empty((B, H), np.float32)
    for c in range(NCORES):
        r = res.results[c]
        dall = np.asarray(r["doc_out"], np.float32).reshape(EPB, D, H)
        qall = np.asarray(r["q_out"], np.float32)
        for k, e in enumerate(ex_map[c]):
            doc[e] = dall[k]
            qp[e] = qall[k]
    q_bcast = np.broadcast_to(qp[:, None, :], (B, D, H))
    return doc, q_bcast
